# revision 16
# baseline (speedup 1.0000x reference)
"""Trainium2 Bass kernel for the conditioned-user-state Euler-ODE MLP.

Computation (per reference, h=1; note 1/0.05 == 20.0 exactly in fp64, so
floor(h/dt) = 20 and the rest-interval branch ALSO fires):
    for 20 steps:  s += 0.05 * mlp(cat(s, u))
    final step:    s += 1.0  * mlp(cat(s, u))        (unscaled rest-interval)
    mlp(x) = silu(silu(x@W1+b1)@W2+b2)@W3+b3

Strategy: pure data-parallel over 8 NeuronCores (1024 batch rows each).
All weights + the state shard live in SBUF for the whole kernel; the 20
sequential MLP applications run entirely on-chip. Activations are kept
feature-major ([feature, batch]) so every matmul is
    psum[m,N] += W[k,m].T @ xT[k,N]
with K=M=128, N=512 (one fp32 PSUM bank).

Matmuls run in float32r (the PE's full-rate fp32 mode, e8m11 — 4x the
throughput of plain fp32). The walrus verifier requires every matmul
operand's producer to emit fp32r, so: weights / user-params are
pre-rounded host-side and DMA'd as fp32r; hidden activations are written
as fp32r directly by the ACT engine's fused silu+bias; the running state
is accumulated in exact fp32 (DVE scalar_tensor_tensor: s = (psum +
b3*dt) + s, with W3/b3 pre-scaled by dt host-side) and a rounded fp32r
shadow copy (sTr) is refreshed each step for the layer-1 matmuls.
"""

import math
import os
import sys

import numpy as np

for _p in ("/opt/trn_rl_repo", "/root/.axon_site/_ro/trn_rl_repo"):
    if os.path.isdir(_p) and _p not in sys.path:
        sys.path.insert(0, _p)

MIN_INTEGRAL = 1e-05
INTERVAL_TIME = 0.05

N_CORES = 8
B, S, U, HID = 8192, 256, 128, 512
P = 128                     # partitions
NT = 512                    # matmul moving free dim (one fp32 PSUM bank)
B_LOC = B // N_CORES        # 1024 rows per core
N_TILES = B_LOC // NT       # 2
KC1, MC1 = (S + U) // P, HID // P   # 3, 4
KC2, MC2 = HID // P, HID // P       # 4, 4
KC3, MC3 = HID // P, S // P         # 4, 2

_NC_CACHE = {}


def _build(
    num_dt_steps: int,
    final_step: bool,
    staggered_reset: bool = False,
    unroll: int = 1,
):
    """Build + compile the Bass program (one program, run SPMD on 8 cores).

    The step loop is a hardware For_i loop: on this platform, device time
    scales with STATIC program size (instruction-stream fetch), so the
    step body must exist once. The per-step Euler scale (dt for all steps,
    1.0 for the final rest-interval step) is read dynamically from a small
    SBUF table indexed by the loop variable.
    """
    import concourse.bass as bass
    import concourse.mybir as mybir
    import concourse.tile as tile
    from concourse import bacc

    f32 = mybir.dt.float32
    f32r = mybir.dt.float32r
    AF = mybir.ActivationFunctionType
    ALU = mybir.AluOpType
    ds = bass.ds

    n_steps = num_dt_steps + (1 if final_step else 0)
    # smalls layout: [b1 (MC1) | b2 (MC2) | b3 (MC3) | scales (n_steps)]
    SM_B1, SM_B2, SM_B3, SM_SC = 0, MC1, MC1 + MC2, MC1 + MC2 + MC3
    sm_cols = SM_SC + n_steps

    nc = bacc.Bacc("TRN2", target_bir_lowering=False, debug=False)

    sT_d = nc.dram_tensor("sT", [P, MC3 * B_LOC], f32, kind="ExternalInput")
    sTr_d = nc.dram_tensor("sTr", [P, MC3 * B_LOC], f32r, kind="ExternalInput")
    uT_d = nc.dram_tensor("uT", [P, B_LOC], f32r, kind="ExternalInput")
    w1_d = nc.dram_tensor("w1p", [P, KC1 * HID], f32r, kind="ExternalInput")
    w2_d = nc.dram_tensor("w2p", [P, KC2 * HID], f32r, kind="ExternalInput")
    w3_d = nc.dram_tensor("w3p", [P, KC3 * S], f32r, kind="ExternalInput")
    sm_d = nc.dram_tensor("smalls", [P, sm_cols], f32, kind="ExternalInput")
    out_d = nc.dram_tensor("outT", [P, MC3 * B_LOC], f32, kind="ExternalOutput")

    with tile.TileContext(nc) as tc:
        with (
            tc.tile_pool(name="const", bufs=1) as cpool,
            tc.tile_pool(name="state", bufs=1) as spool,
            tc.tile_pool(name="acts", bufs=2) as apool,
            tc.tile_pool(name="psum", bufs=8, space="PSUM") as ppool,
        ):
            sT = spool.tile([P, MC3 * B_LOC], f32, name="sT_sb")
            sTr = spool.tile([P, MC3 * B_LOC], f32r, name="sTr_sb")
            uT = cpool.tile([P, B_LOC], f32r, name="uT_sb")
            w1 = cpool.tile([P, KC1 * HID], f32r, name="w1_sb")
            w2 = cpool.tile([P, KC2 * HID], f32r, name="w2_sb")
            w3 = cpool.tile([P, KC3 * S], f32r, name="w3_sb")
            sm = cpool.tile([P, sm_cols], f32, name="sm_sb")

            # Spread the one-time loads over several DGE queues.
            nc.sync.dma_start(sTr[:], sTr_d[:])
            nc.sync.dma_start(uT[:], uT_d[:])
            nc.scalar.dma_start(w1[:], w1_d[:])
            nc.gpsimd.dma_start(w2[:], w2_d[:])
            nc.scalar.dma_start(w3[:], w3_d[:])
            nc.sync.dma_start(sT[:], sT_d[:])
            nc.sync.dma_start(sm[:], sm_d[:])

            def x1_chunk(k, n):
                """Layer-1 rhs chunk k of cat(state, user), fp32r."""
                if k < MC3:
                    return sTr[:, k * B_LOC + n * NT : k * B_LOC + (n + 1) * NT]
                return uT[:, n * NT : (n + 1) * NT]

            def emit_step(sc_ap):
                h1s, h2s = [], []
                for n in range(N_TILES):
                    h1 = apool.tile([P, MC1 * NT], f32r, tag="h1", name="h1")
                    for m in range(MC1):
                        ps = ppool.tile([P, NT], f32, tag="ps", name="ps")
                        for k in range(KC1):
                            nc.tensor.matmul(
                                ps[:],
                                w1[:, k * HID + m * P : k * HID + (m + 1) * P],
                                x1_chunk(k, n),
                                start=(k == 0),
                                stop=(k == KC1 - 1),
                            )
                        nc.scalar.activation(
                            h1[:, m * NT : (m + 1) * NT], ps[:], AF.Silu,
                            bias=sm[:, SM_B1 + m : SM_B1 + m + 1],
                        )
                    h1s.append(h1)

                for n in range(N_TILES):
                    h1 = h1s[n]
                    h2 = apool.tile([P, MC2 * NT], f32r, tag="h2", name="h2")
                    for m in range(MC2):
                        ps = ppool.tile([P, NT], f32, tag="ps", name="ps")
                        for k in range(KC2):
                            nc.tensor.matmul(
                                ps[:],
                                w2[:, k * HID + m * P : k * HID + (m + 1) * P],
                                h1[:, k * NT : (k + 1) * NT],
                                start=(k == 0),
                                stop=(k == KC2 - 1),
                            )
                        nc.scalar.activation(
                            h2[:, m * NT : (m + 1) * NT], ps[:], AF.Silu,
                            bias=sm[:, SM_B2 + m : SM_B2 + m + 1],
                        )
                    h2s.append(h2)

                for n in range(N_TILES):
                    h2 = h2s[n]
                    for m in range(MC3):
                        ps = ppool.tile([P, NT], f32, tag="ps", name="ps")
                        for k in range(KC3):
                            nc.tensor.matmul(
                                ps[:],
                                w3[:, k * S + m * P : k * S + (m + 1) * P],
                                h2[:, k * NT : (k + 1) * NT],
                                start=(k == 0),
                                stop=(k == KC3 - 1),
                            )
                        lo = m * B_LOC + n * NT
                        hi = lo + NT
                        # t = (psum + b3) * sc ; s += t ; sTr = round(s)
                        t = apool.tile([P, NT], f32, tag="t", name="t")
                        nc.vector.tensor_scalar(
                            t[:], ps[:],
                            sm[:, SM_B3 + m : SM_B3 + m + 1], sc_ap,
                            ALU.add, ALU.mult,
                        )
                        nc.vector.tensor_add(sT[:, lo:hi], sT[:, lo:hi], t[:])
                        nc.vector.tensor_copy(sTr[:, lo:hi], sT[:, lo:hi])

            if n_steps % unroll != 0:
                unroll = 1
            with tc.For_i(0, n_steps, unroll, staggered_reset=staggered_reset) as iv:
                for j in range(unroll):
                    emit_step(sm[:, ds(SM_SC + iv + j, 1)])

            nc.sync.dma_start(out_d[:], sT[:])

    nc.compile()
    return nc


def _get_nc(num_dt_steps: int, final_step: bool):
    key = (num_dt_steps, final_step)
    if key not in _NC_CACHE:
        _NC_CACHE[key] = _build(num_dt_steps, final_step)
    return _NC_CACHE[key]


def _round_fp32r(x: np.ndarray) -> np.ndarray:
    """Round fp32 to fp32r (e8m11): round-to-nearest-even to 11 mantissa bits."""
    b = np.ascontiguousarray(x, dtype=np.float32).view(np.uint32)
    shift = np.uint32(12)
    one = np.uint32(1)
    bias = ((b >> shift) & one) + np.uint32((1 << 11) - 1)
    r = (b + bias) & np.uint32(0xFFFFFFFF ^ ((1 << 12) - 1))
    return r.view(np.float32)


def _pack_feature_major(x_bf: np.ndarray, n_chunks: int, ncols: int) -> np.ndarray:
    """[batch, feat] -> [128, n_chunks*ncols] feature-chunk packing."""
    xt = np.ascontiguousarray(x_bf.T, dtype=np.float32)  # [feat, cols]
    return np.ascontiguousarray(
        xt.reshape(n_chunks, P, ncols).transpose(1, 0, 2).reshape(P, n_chunks * ncols)
    )


def _pack_weight(w: np.ndarray, kc: int, mdim: int) -> np.ndarray:
    """[K, M] -> [128, kc*M]; chunk k at cols [k*M:(k+1)*M]."""
    return np.ascontiguousarray(
        np.asarray(w, dtype=np.float32).reshape(kc, P, mdim).transpose(1, 0, 2)
        .reshape(P, kc * mdim)
    )


def _pack_bias(b: np.ndarray, mc: int) -> np.ndarray:
    """[M] -> [128, mc]; col m holds b[m*128:(m+1)*128]."""
    return np.ascontiguousarray(
        np.asarray(b, dtype=np.float32).reshape(mc, P).T
    )


def _prepare_in_maps(state, user_params, W1, b1, W2, b2, W3, b3, n_steps, final_step):
    state = np.ascontiguousarray(np.asarray(state), dtype=np.float32)
    user_params = np.ascontiguousarray(np.asarray(user_params), dtype=np.float32)
    W1 = np.asarray(W1, dtype=np.float32)
    b1 = np.asarray(b1, dtype=np.float32)
    W2 = np.asarray(W2, dtype=np.float32)
    b2 = np.asarray(b2, dtype=np.float32)
    W3 = np.asarray(W3, dtype=np.float32)
    b3 = np.asarray(b3, dtype=np.float32)

    scales = np.full(n_steps, INTERVAL_TIME, dtype=np.float32)
    if final_step:
        scales[-1] = 1.0
    smalls = np.concatenate(
        [_pack_bias(b1, MC1), _pack_bias(b2, MC2), _pack_bias(b3, MC3),
         np.broadcast_to(scales, (P, n_steps))],
        axis=1,
    )
    shared = {
        "w1p": _round_fp32r(_pack_weight(W1, KC1, HID)),
        "w2p": _round_fp32r(_pack_weight(W2, KC2, HID)),
        "w3p": _round_fp32r(_pack_weight(W3, KC3, S)),
        "smalls": np.ascontiguousarray(smalls, dtype=np.float32),
    }
    in_maps = []
    for c in range(N_CORES):
        rows = slice(c * B_LOC, (c + 1) * B_LOC)
        sT_c = _pack_feature_major(state[rows], MC3, B_LOC)
        in_maps.append(
            {
                "sT": sT_c,
                "sTr": _round_fp32r(sT_c),
                "uT": _round_fp32r(
                    np.ascontiguousarray(user_params[rows].T, dtype=np.float32)
                ),
                **shared,
            }
        )
    return in_maps


def _exec(nc, in_maps, trace=False):
    import time

    from concourse.bass_utils import run_bass_kernel_spmd

    try:
        return run_bass_kernel_spmd(
            nc, in_maps, core_ids=list(range(N_CORES)), trace=trace
        )
    except Exception:
        # The axon-tunneled devices occasionally throw transient
        # NRT_EXEC_UNIT_UNRECOVERABLE; one retry usually succeeds.
        time.sleep(2.0)
        return run_bass_kernel_spmd(
            nc, in_maps, core_ids=list(range(N_CORES)), trace=trace
        )


def _unpack_output(results):
    outs = []
    for c in range(N_CORES):
        o = np.asarray(results[c]["outT"])  # [128, MC3*B_LOC]
        s_new = (
            o.reshape(P, MC3, B_LOC).transpose(1, 0, 2).reshape(MC3 * P, B_LOC).T
        )
        outs.append(s_new)
    return np.ascontiguousarray(np.concatenate(outs, axis=0), dtype=np.float32)


def _run(state, user_params, W1, b1, W2, b2, W3, b3, h, trace=False):
    hf = float(h)
    num_intervals = int(math.floor(hf / INTERVAL_TIME))
    rest = hf % INTERVAL_TIME
    final_step = bool(MIN_INTEGRAL < rest)
    if num_intervals == 0 and not final_step:
        return (
            np.ascontiguousarray(np.asarray(state), dtype=np.float32).copy(),
            None,
        )

    nc = _get_nc(num_intervals, final_step)
    n_steps = num_intervals + (1 if final_step else 0)
    in_maps = _prepare_in_maps(
        state, user_params, W1, b1, W2, b2, W3, b3, n_steps, final_step
    )
    res = _exec(nc, in_maps, trace=trace)
    full = _unpack_output(res.results)
    return full, getattr(res, "exec_time_ns", None)


def kernel(**inputs) -> np.ndarray:
    out, _ = _run(**inputs)
    return out


def run_with_timing(trace=False, **inputs):
    return _run(**inputs, trace=trace)
